# revision 29
# baseline (speedup 1.0000x reference)
"""Trainium2 Bass kernel for causal self-attention with RoPE.

Model: x[4,2048,1024] -> qkv = x@Wqkv -> RoPE(q,k) -> causal SDPA -> out@Wout.

Sharding (8 cores): core c handles batch b=c//2, head-group g=c%2 (8 of 16
heads).  Each core computes a partial output: x[b] attention restricted to its
heads, projected through its slice of Wout rows; the host sums the two
partials per batch.

v2 layout strategy (cost model: matmul charges only moving columns; bf16
unlocks DVE 2x; Pool engine absorbs evictions):
  - all matmul operands bf16 (x, W, qT/kT, V, exp(scores), attnT, Wout);
    PSUM accumulation stays fp32.
  - qT/kT produced in [head_dim, tok] layout with fused RoPE (DVE 2x bf16).
  - scoresT[k,q] = kT.T @ qT in pair tiles [128, 2*512]; exp on ScalarE
    (no max subtraction; scores bounded), causal mask via bf16 multiply
    with a triangular mask on diagonal tiles only (DVE 2x).
  - A@V flipped vs v1: stationary = at[k, q-tile] (full 128-wide),
    moving = V_aug[k, 65] (ones column -> row-sum in col 64) ->
    out PSUM [q, 65] accumulated over k-tiles: 65 cycles per (q-tile,
    k-tile) instead of 512 per k-tile.
  - normalization: reciprocal of col 64 (free-size 4 per head-span) and one
    fused multiply-evict to attn_sb[q, tt, h, hd] bf16.  No PE broadcast.
  - attn stays in SBUF (no DRAM bounce); per token-tile, 8 cheap PE
    transposes (bf16) assemble attnT [feat,128] for the output projection.
  - evictions distributed: V/attnT tiles on Pool (was idle), qkv/score
    paths on DVE, exp exclusively on ACT.
"""

import os
import sys

import numpy as np


def _import_concourse():
    try:
        import concourse  # noqa: F401
    except ImportError:
        for p in ("/opt/trn_rl_repo", "/root/.axon_site/_ro/trn_rl_repo"):
            if os.path.isdir(p) and p not in sys.path:
                sys.path.insert(0, p)
        import concourse  # noqa: F401


_import_concourse()

import concourse.bacc as bacc
import concourse.bass as bass
import concourse.mybir as mybir
import concourse.tile as tile
from concourse.masks import make_identity
from concourse.bass_utils import run_bass_kernel_spmd

# ---------------------------------------------------------------------------
# Problem constants (hardcoded per the harness contract).
D_MODEL = 1024
N_HEADS = 16
HEAD_DIM = 64
ROPE_BASE = 10000.0
BATCH = 4
T_FULL = 2048
N_CORES = 8

HPC = 8                 # heads per core
FEAT = HPC * HEAD_DIM   # 512 = per-core q/k/v feature width
DCH = D_MODEL // 128    # 8 contraction chunks of 128

F32 = mybir.dt.float32
BF16 = mybir.dt.bfloat16

CFG = {
    "tag": "v3",
    # fp8e4m3 + DoubleRow for the V projection only (4x PE rate there).
    # q/k stay bf16 (fp8 logit noise breaks peaked-attention rows).  Wv is
    # pre-scaled x32 on the host (dodges fp8 subnormals); compensated by a
    # 32.0 ones-column (the denominator picks up the same x32 as A@V).
    "v_fp8": False,
}
W_SCALE = 32.0


def build_nc(T=T_FULL, cfg=CFG):
    """Build the per-core Bass program (SPMD: same program on all cores)."""
    SPAN1 = 256 if T >= 256 else T          # phase-1 token span
    NSPAN1 = T // SPAN1
    SPAN2 = 512 if T >= 512 else T          # attention q span
    NSPAN2 = T // SPAN2
    NTOK = T // 128
    KT_PER_SPAN = SPAN2 // 128
    NFB = FEAT // 128

    nc = bacc.Bacc(None, target_bir_lowering=False)

    V_FP8 = bool(cfg.get("v_fp8"))
    FP8 = mybir.dt.float8e4
    VDT = FP8 if V_FP8 else BF16
    xt_d = nc.dram_tensor("xt", [D_MODEL, T], BF16, kind="ExternalInput")
    if V_FP8:
        xt8_d = nc.dram_tensor("xt8", [D_MODEL, T], FP8, kind="ExternalInput")
    wq_d = nc.dram_tensor("wq", [D_MODEL, FEAT], BF16, kind="ExternalInput")
    wk_d = nc.dram_tensor("wk", [D_MODEL, FEAT], BF16, kind="ExternalInput")
    wv_d = nc.dram_tensor("wv", [D_MODEL, FEAT], VDT, kind="ExternalInput")
    wo_d = nc.dram_tensor("wo", [FEAT, D_MODEL], BF16, kind="ExternalInput")
    cs_d = nc.dram_tensor("cs", [128, T], BF16, kind="ExternalInput")
    sn_d = nc.dram_tensor("sn", [128, T], BF16, kind="ExternalInput")
    mk_d = nc.dram_tensor("mk", [128, 128], BF16, kind="ExternalInput")
    out_d = nc.dram_tensor("out", [T, D_MODEL], F32, kind="ExternalOutput")
    DEBUG = bool(cfg.get("debug"))
    if DEBUG:
        NTOK_ = T // 128
        dbgq_d = nc.dram_tensor("dbgq", [128, (FEAT // 128) * T], F32,
                                kind="ExternalOutput")
        dbgk_d = nc.dram_tensor("dbgk", [128, (FEAT // 128) * T], F32,
                                kind="ExternalOutput")
        dbgv_d = nc.dram_tensor("dbgv", [128, NTOK_ * HPC * (HEAD_DIM + 1)],
                                F32, kind="ExternalOutput")
        dbga_d = nc.dram_tensor("dbga", [128, NTOK_ * HPC * HEAD_DIM], F32,
                                kind="ExternalOutput")
        dbgt_d = nc.dram_tensor("dbgt", [128, NTOK_, FEAT // 128, 128], BF16,
                                kind="ExternalOutput")

    with tile.TileContext(nc) as tc:
        pools = []

        def pool(name, bufs, space="SBUF"):
            p = tc.alloc_tile_pool(name=name, bufs=bufs, space=space)
            pools.append(p)
            return p

        def release(*ps):
            for p in reversed(ps):
                assert p is pools[-1]
                p.release()
                pools.pop()

        # ---- persistent tensors --------------------------------------
        pbig = pool("big", 1)
        qT = pbig.tile([128, NFB, T], BF16, name="qT")
        kT = pbig.tile([128, NFB, T], BF16, name="kT")
        v_sb = pbig.tile([128, NTOK, HPC, HEAD_DIM + 1], BF16, name="v_sb")
        attn_sb = pbig.tile([128, NTOK, HPC, HEAD_DIM], BF16, name="attn_sb")
        mk_sb = pbig.tile([128, 128], BF16, name="mk_sb")
        id_sb = pbig.tile([128, 128], BF16, name="id_sb")
        make_identity(nc, id_sb[:])
        # ones column of V_aug (softmax denominator trick); W_SCALE when the
        # qkv weights are pre-scaled so the denominator matches A@V's scale.
        nc.gpsimd.memset(v_sb[:, :, :, HEAD_DIM],
                         W_SCALE if V_FP8 else 1.0)

        # ---- attention pools (PSUM first so banks are disjoint from the
        # phase-1 qkv pool and the phases overlap) ------------------------
        p2s = pool("p2s", 2, space="PSUM")   # score pairs [128,2*SPAN2]: 4 banks
        p2a = pool("p2a", 2, space="PSUM")   # A@V accum [128,512]f32: 2 banks
        p2at = pool("p2at", 16)              # exp(scores) bf16
        p2rc = pool("p2rc", 4)               # reciprocal of row sums

        # ---- phase-1 pools ----------------------------------------------
        p1w = pool("p1w", 1)
        p1x = pool("p1x", 2)
        p1t = pool("p1t", 2)
        p1c = pool("p1c", 2)
        p1ps = pool("p1ps", 2, space="PSUM")  # shared qkv-accum tag: 2 banks

        DC2 = DCH // 2 if V_FP8 else DCH
        vshape = [128, DC2, 2, FEAT] if V_FP8 else [128, DCH, FEAT]
        wq_sb = p1w.tile([128, DCH, FEAT], BF16, name="wq_sb")
        wk_sb = p1w.tile([128, DCH, FEAT], BF16, name="wk_sb")
        wv_sb = p1w.tile(vshape, VDT, name="wv_sb")

        xt_view = xt_d[:].rearrange("(c p) t -> p c t", p=128)
        if V_FP8:
            xt8_view = xt8_d[:].rearrange("(c i p) t -> p c i t", p=128, i=2)
            wv_view = wv_d[:].rearrange("(c i p) f -> p c i f", p=128, i=2)
        else:
            wv_view = wv_d[:].rearrange("(c p) f -> p c f", p=128)
        wq_v = wq_d[:].rearrange("(c p) f -> p c f", p=128)
        wk_v = wk_d[:].rearrange("(c p) f -> p c f", p=128)

        # DMA issue order matches need order.
        xt0 = p1x.tile([128, DCH, SPAN1], BF16, tag="xt")
        nc.sync.dma_start(xt0[:], xt_view[:, :, 0:SPAN1])
        if V_FP8:
            xt80 = p1x.tile([128, DC2, 2, SPAN1], FP8, tag="xt8")
            nc.sync.dma_start(xt80[:], xt8_view[:, :, :, 0:SPAN1])
        for fb in range(NFB):
            nc.sync.dma_start(wq_sb[:, :, fb * 128:(fb + 1) * 128],
                              wq_v[:, :, fb * 128:(fb + 1) * 128])
        for fb in range(NFB):
            nc.sync.dma_start(wk_sb[:, :, fb * 128:(fb + 1) * 128],
                              wk_v[:, :, fb * 128:(fb + 1) * 128])
        nc.sync.dma_start(wv_sb[:], wv_view)
        nc.sync.dma_start(mk_sb[:], mk_d[:])

        def p1_gen(s1):
            """qkv projection + RoPE for one SPAN1 token span, as a
            generator of emission units (for weaving into attention)."""
            sl = slice(s1 * SPAN1, (s1 + 1) * SPAN1)
            if s1 == 0:
                xt = xt0
                xt8 = xt80 if V_FP8 else None
            else:
                xt = p1x.tile([128, DCH, SPAN1], BF16, tag="xt")
                nc.sync.dma_start(xt[:], xt_view[:, :, sl])
                if V_FP8:
                    xt8 = p1x.tile([128, DC2, 2, SPAN1], FP8, tag="xt8")
                    nc.sync.dma_start(xt8[:], xt8_view[:, :, :, sl])
            cs_sp = p1c.tile([128, SPAN1], BF16, tag="cs")
            sn_sp = p1c.tile([128, SPAN1], BF16, tag="sn")
            nc.sync.dma_start(cs_sp[:], cs_d[:, sl])
            nc.sync.dma_start(sn_sp[:], sn_d[:, sl])
            yield
            csb = bass.AP(cs_sp.tensor, cs_sp.offset,
                          [cs_sp.ap[0], [0, NFB], cs_sp.ap[1]])
            snb = bass.AP(sn_sp.tensor, sn_sp.offset,
                          [sn_sp.ap[0], [0, NFB], sn_sp.ap[1]])
            # qT / kT with fused RoPE: 4 feature blocks evicted (DVE) into
            # one [128, 4, SPAN1] bf16 tile, rotate-half via 4 SBUF->SBUF
            # DMAs, RoPE itself is 3 full-width DVE ops at bf16 2x.
            for wsb, dst in ((wq_sb, qT), (wk_sb, kT)):
                qr = p1t.tile([128, NFB, SPAN1], BF16, tag="qr")
                for fb in range(NFB):
                    ps = p1ps.tile([128, SPAN1], F32, tag="p1ps")
                    for c in range(DCH):
                        nc.tensor.matmul(
                            ps[:],
                            wsb[:, c, fb * 128:(fb + 1) * 128],
                            xt[:, c, :],
                            start=(c == 0),
                            stop=(c == DCH - 1),
                        )
                    nc.vector.tensor_copy(qr[:, fb, :], ps[:])
                    yield
                qs = p1t.tile([128, NFB, SPAN1], BF16, tag="qs")
                for r0, sr in ((0, 32), (32, 0), (64, 96), (96, 64)):
                    nc.sync.dma_start(qs[r0:r0 + 32, :, :], qr[sr:sr + 32, :, :])
                nc.vector.tensor_mul(qs[:], qs[:], snb)
                nc.vector.tensor_mul(qr[:], qr[:], csb)
                nc.vector.tensor_add(dst[:, :, sl], qr[:], qs[:])
                yield
            # V in natural [tok, feat] layout (evicted on Pool)
            for tt in range(SPAN1 // 128):
                ktile = s1 * (SPAN1 // 128) + tt
                pv = p1ps.tile([128, FEAT], F32, tag="p1ps")
                for c in range(DC2):
                    if V_FP8:
                        nc.tensor.matmul(
                            pv[:], xt8[:, c, :, tt * 128:(tt + 1) * 128],
                            wv_sb[:, c, :, :],
                            start=(c == 0), stop=(c == DC2 - 1),
                            perf_mode=mybir.MatmulPerfMode.DoubleRow,
                        )
                    else:
                        nc.tensor.matmul(
                            pv[:], xt[:, c, tt * 128:(tt + 1) * 128],
                            wv_sb[:, c, :],
                            start=(c == 0), stop=(c == DC2 - 1),
                        )
                nc.vector.tensor_copy(
                    v_sb[:, ktile, :, 0:HEAD_DIM],
                    pv[:].rearrange("p (h d) -> p h d", d=HEAD_DIM),
                )
                yield

        # ---- attention span machinery -----------------------------------
        TRI = slice(0, 128)

        def lo_of(s, j):
            return max(0, (j - s * KT_PER_SPAN) * 128)

        def produce(pairs, at_buf, idx):
            h, s, ja, jmax = pairs[idx]
            hrow = 64 * (h % 2)
            hc = h // 2
            ps = p2s.tile([128, 2 * SPAN2], F32, tag="ps_s")
            at = p2at.tile([128, 2 * SPAN2], BF16, tag="at")
            lo_a = lo_of(s, ja)
            for half, j in enumerate((ja, ja + 1)):
                base = half * SPAN2
                # half A computes from the pair's lower bound; half B its
                # full range, so one exp over [lo_a:) sees no uninitialized
                # gap (extra columns are never consumed by the A@V matmul).
                lo = lo_a if half == 0 else 0
                nc.tensor.matmul(
                    ps[:, base + lo:base + SPAN2],
                    kT[hrow:hrow + 64, hc, j * 128:(j + 1) * 128],
                    qT[hrow:hrow + 64, hc, s * SPAN2 + lo:(s + 1) * SPAN2],
                    start=True,
                    stop=True,
                )
            sc = 1.0 / np.sqrt(HEAD_DIM)
            nc.scalar.activation(
                at[:, lo_a:], ps[:, lo_a:],
                mybir.ActivationFunctionType.Exp,
                scale=float(sc),
            )
            j0 = s * KT_PER_SPAN
            for half, j in enumerate((ja, ja + 1)):
                if j >= j0:  # diagonal tile: mask the [128,128] triangle
                    jp = j - j0
                    tb = half * SPAN2 + jp * 128
                    nc.gpsimd.tensor_mul(
                        at[:, tb:tb + 128], at[:, tb:tb + 128], mk_sb[:, TRI]
                    )
            at_buf[idx] = at

        def attn_span(s, weave=None, k=1):
            """All heads of q-span s.  Per head: all score pairs are produced
            first (pipelined one head ahead), then the A@V runs one q-tile
            accumulation group at a time (PSUM zeroing is per 2KB bank, so
            concurrent groups must not share a bank).  After each unit, up to
            `k` items are drawn from the `weave` iterator."""
            jmax = (s + 1) * KT_PER_SPAN - 1
            npairs = (jmax + 1) // 2
            pairs = []
            for h in range(HPC):
                for ja in range(0, jmax + 1, 2):
                    pairs.append((h, s, ja, jmax))
            at_buf = {}
            nprod = 0
            j0 = s * KT_PER_SPAN

            def prod_upto(tgt):
                nonlocal nprod
                while nprod < min(tgt, len(pairs)):
                    produce(pairs, at_buf, nprod)
                    nprod += 1

            def wv():
                if weave is not None:
                    for _ in range(k):
                        next(weave, None)

            for h in range(HPC):
                # pairs for this head fully produced; next head's trickle in
                prod_upto((h + 1) * npairs)
                ats = [at_buf.pop(h * npairs + i) for i in range(npairs)]
                for qt in range(KT_PER_SPAN):
                    aps = p2a.tile([128, 512], F32, tag="ps_a")
                    prod_upto((h + 1) * npairs + (qt + 1) * 2)
                    wv()
                    for j in range(0, j0 + qt + 1):
                        at = ats[j // 2]
                        base = (j % 2) * SPAN2
                        nc.tensor.matmul(
                            aps[:, 0:HEAD_DIM + 1],
                            at[:, base + qt * 128:base + (qt + 1) * 128],
                            v_sb[:, j, h, :],
                            start=(j == 0),
                            stop=(j == j0 + qt),
                        )
                    # evict: reciprocal of the ones-column row sum, then one
                    # fused normalize-evict into attn_sb (bf16).
                    rc = p2rc.tile([128, 1], F32, tag="rc")
                    nc.vector.reciprocal(rc[:], aps[:, HEAD_DIM:HEAD_DIM + 1])
                    rcb = bass.AP(rc.tensor, rc.offset,
                                  [rc.ap[0], [0, HEAD_DIM]])
                    nc.vector.tensor_mul(
                        attn_sb[:, s * KT_PER_SPAN + qt, h, :],
                        aps[:, 0:HEAD_DIM],
                        rcb,
                    )

        # ---- interleaved schedule ---------------------------------------
        from itertools import chain as _chain

        def run_gen(g):
            for _ in g:
                pass

        run_gen(p1_gen(0))
        run_gen(p1_gen(1))
        for seg in range(NSPAN2 - 1):
            nxt = 2 * seg + 2
            if nxt < NSPAN1:
                w = _chain(p1_gen(nxt), p1_gen(nxt + 1))
            else:
                w = iter(())
            attn_span(seg, weave=w, k=2 if seg == 0 else 1)
            for _ in w:
                pass

        release(p1w, p1x, p1t, p1c, p1ps)

        # ---- output projection pools (reuse freed phase-1 space) --------
        p3w = pool("p3w", 1)
        p3at = pool("p3at", 2)
        p3o = pool("p3o", 2)
        p3t = pool("p3t", 1, space="PSUM")   # transposed attn [128,4,128]bf16
        p3p = pool("p3p", 1, space="PSUM")   # proj accum [128,512]f32
        wo_sb = p3w.tile([128, NFB, D_MODEL], BF16, name="wo_sb")
        nc.sync.dma_start(wo_sb[:], wo_d[:].rearrange("(c p) d -> p c d", p=128))

        def emit_proj_tt(tt):
            # One transpose per head-pair: stationary [128, 2*64] -> out
            # [128 feat, 128 q], exactly the projection chunk layout.  All 4
            # form ONE psum accumulation group (the bank is zeroed by the
            # first start; later writes land additively in disjoint regions).
            psT = p3t.tile([128, NFB, 128], BF16, tag="psT")
            for c in range(NFB):
                nc.tensor.matmul(
                    psT[:, c, :],
                    attn_sb[:, tt, 2 * c:2 * c + 2, :],
                    id_sb[:],
                    is_transpose=True,
                    start=(c == 0),
                    stop=(c == NFB - 1),
                    skip_group_check=True,
                )
            aT = p3at.tile([128, NFB, 128], BF16, tag="aT")
            nc.vector.tensor_copy(aT[:], psT[:])
            if DEBUG:
                nc.sync.dma_start(dbgt_d[:, tt], aT[:])
            for ns in range(D_MODEL // 512):
                po = p3p.tile([128, 512], F32, tag="ps_o")
                for c in range(NFB):
                    nc.tensor.matmul(
                        po[:],
                        aT[:, c, :],
                        wo_sb[:, c, ns * 512:(ns + 1) * 512],
                        start=(c == 0),
                        stop=(c == NFB - 1),
                    )
                ot = p3o.tile([128, 512], F32, tag="ot")
                nc.vector.tensor_copy(ot[:], po[:])
                nc.sync.dma_start(
                    out_d[tt * 128:(tt + 1) * 128, ns * 512:(ns + 1) * 512],
                    ot[:],
                )

        # last attention span, with the projection of the earlier spans
        # woven in one token-tile per few pairs
        def proj_gen(tts):
            for tt in tts:
                emit_proj_tt(tt)
                yield

        early = list(range((NSPAN2 - 1) * KT_PER_SPAN))
        npairs_last = HPC * (NSPAN2 * KT_PER_SPAN) // 2
        stride = max(1, npairs_last // max(1, len(early)))

        class _Paced:
            def __init__(self, gen, stride):
                self.gen, self.stride, self.n = gen, stride, 0

            def __next__(self):
                self.n += 1
                if self.n % self.stride == 0:
                    return next(self.gen, None)
                return None

        paced = _Paced(proj_gen(early), stride)
        attn_span(NSPAN2 - 1, weave=paced, k=1)
        for _ in paced.gen:  # drain any remaining early token-tiles
            pass
        for tt in range((NSPAN2 - 1) * KT_PER_SPAN, NTOK):
            emit_proj_tt(tt)

        if DEBUG:
            pdbg = pool("pdbg", 1)
            for nm, src, dst in (
                ("q", qT, dbgq_d), ("k", kT, dbgk_d),
                ("v", v_sb, dbgv_d), ("a", attn_sb, dbga_d),
            ):
                n = 1
                for d in src.shape[1:]:
                    n *= d
                t32 = pdbg.tile([128, n], F32, name=f"dbg_{nm}")
                flat = bass.AP(src.tensor, src.offset,
                               [src.ap[0], [1, n]])
                nc.vector.tensor_copy(t32[:], flat)
                nc.sync.dma_start(dst[:], t32[:])
            pdbg.release()
            pools.pop()

        for p in reversed(pools):
            p.release()
        pools.clear()

    nc.finalize()
    return nc


# ---------------------------------------------------------------------------
# Host-side input prep


def _bf16():
    import ml_dtypes

    return ml_dtypes.bfloat16


def rope_tables(T):
    inv_freq = 1.0 / (
        ROPE_BASE ** (np.arange(0, HEAD_DIM, 2, dtype=np.float64) / HEAD_DIM)
    )
    freqs = np.arange(T, dtype=np.float64)[:, None] * inv_freq[None, :]  # [T, 32]
    emb = np.concatenate([freqs, freqs], axis=-1)  # [T, 64]
    cos = np.cos(emb).T  # [64, T]
    sin = np.sin(emb).T
    bf = _bf16()
    cs = np.tile(cos, (2, 1)).astype(bf)  # [128, T]
    sn_half = np.concatenate([-sin[:32], sin[32:]], axis=0)  # [64, T] signed
    sn = np.tile(sn_half, (2, 1)).astype(bf)
    return np.ascontiguousarray(cs), np.ascontiguousarray(sn)


def _fp8():
    import ml_dtypes

    return ml_dtypes.float8_e4m3


def make_core_inputs(x, Wqkv, Wout, T=T_FULL, cfg=CFG):
    bf = _bf16()
    v8 = cfg.get("v_fp8")
    cs, sn = rope_tables(T)
    u = np.arange(128)[None, :]
    p = np.arange(128)[:, None]
    mk = (u >= p).astype(bf)

    in_maps = []
    for core in range(N_CORES):
        b, g = divmod(core, 2)
        in_maps.append(
            {
                "xt": np.ascontiguousarray(x[b].T).astype(bf),
                "wq": np.ascontiguousarray(Wqkv[:, g * FEAT:(g + 1) * FEAT]).astype(bf),
                "wk": np.ascontiguousarray(
                    Wqkv[:, D_MODEL + g * FEAT:D_MODEL + (g + 1) * FEAT]
                ).astype(bf),
                "wv": np.ascontiguousarray(
                    (W_SCALE if v8 else 1.0)
                    * Wqkv[:, 2 * D_MODEL + g * FEAT:2 * D_MODEL + (g + 1) * FEAT]
                ).astype(_fp8() if v8 else bf),
                "wo": np.ascontiguousarray(Wout[g * FEAT:(g + 1) * FEAT, :]).astype(bf),
                **({"xt8": np.ascontiguousarray(x[b].T).astype(_fp8())}
                   if v8 else {}),
                "cs": cs,
                "sn": sn,
                "mk": mk,
            }
        )
    return in_maps


_NC_CACHE = {}


def get_nc(T=T_FULL):
    key = (T, tuple(sorted((k, str(v)) for k, v in CFG.items())))
    if key not in _NC_CACHE:
        _NC_CACHE[key] = build_nc(T, CFG)
    return _NC_CACHE[key]


def kernel(x, Wqkv, Wout):
    x = np.asarray(x, dtype=np.float32)
    Wqkv = np.asarray(Wqkv, dtype=np.float32)
    Wout = np.asarray(Wout, dtype=np.float32)
    b, t, _ = x.shape
    assert (b, t) == (BATCH, T_FULL)

    nc = get_nc(T_FULL)
    in_maps = make_core_inputs(x, Wqkv, Wout, T_FULL, CFG)
    res = None
    for attempt in range(3):
        try:
            res = run_bass_kernel_spmd(nc, in_maps, core_ids=list(range(N_CORES)))
            break
        except Exception:
            if attempt == 2:
                raise
            import time

            time.sleep(5.0)
    out = np.empty((BATCH, T_FULL, D_MODEL), dtype=np.float32)
    for bb in range(BATCH):
        out[bb] = res.results[2 * bb]["out"] + res.results[2 * bb + 1]["out"]
    return out
